# revision 21
# baseline (speedup 1.0000x reference)
"""DrugBAN3D Trainium2 kernel — 8-core SPMD Bass/Tile implementation.

Sharding: ligand rows (320) and pocket rows (800) are split 8 ways
(40 / 100 rows per core). Each core computes its rows through
multiscale -> fusion -> cross-attention -> LN -> partial segment sums,
with AllGathers at the two points where full node sets are needed
(projected features for the distance-weighted aggregation; enhanced
features for cross-attention K/V) and one AllReduce for the readout.
The decoder (B=32) is replicated on every core.

Key structural fact used (exact, not approximate): the distance-encoder
MLP de3(relu(de2(relu(de1(D))))) has all-zero biases, and D >= 0, so
mean_H(enc) == alpha * D for a scalar alpha = mean(W3^T relu(W2^T relu(w1))).
The [Nl,Np,H] encoding is never materialized.
"""

import sys

for _p in ("/opt/trn_rl_repo", "/root/.axon_site/_ro/trn_rl_repo"):
    if _p not in sys.path:
        sys.path.append(_p)

import numpy as np

import concourse.bass as bass
import concourse.mybir as mybir
import concourse.tile as tile
from concourse import bacc
from concourse.bass_utils import run_bass_kernel_spmd

F32 = mybir.dt.float32
AF = mybir.ActivationFunctionType
ALU = mybir.AluOpType

NC = 8
NL, NP, H, B = 320, 800, 256, 32
LLOC, PLOC = NL // NC, NP // NC     # 40, 100
HEADS, DH = 8, 32
SCALES = (2.0, 5.0, 8.0)
MLP_HID, MLP_OUT = 512, 128


def _np(x):
    return np.ascontiguousarray(np.asarray(x, dtype=np.float32))


def _chunks(n, c=128):
    out = []
    off = 0
    while off < n:
        out.append((off, min(c, n - off)))
        off += c
    return out


def _bvec(b, parts=128):
    """bias vector [M] -> [128, ceil(M/128)] with b[c*128+p] at [p, c]."""
    b = _np(b)
    m = b.shape[0]
    nc_ = (m + parts - 1) // parts
    out = np.zeros((parts, nc_), np.float32)
    for c in range(nc_):
        seg = b[c * parts:(c + 1) * parts]
        out[: seg.shape[0], c] = seg
    return out


def _prep_host(inputs):
    """Host-side preprocessing: weights dict + per-core inputs + alpha."""
    ms, fp, mp = inputs["ms_params"], inputs["fus_params"], inputs["mlp_params"]
    lig_x, poc_x = _np(inputs["lig_x"]), _np(inputs["poc_x"])
    lc, pc = _np(inputs["lig_coords"]), _np(inputs["poc_coords"])
    lgid, pgid = np.asarray(inputs["lig_gid"]), np.asarray(inputs["poc_gid"])

    def W(p):
        return _np(p["w"])

    def bv(p):
        return _np(p["b"])

    w1 = W(fp["de1"])[0]
    r2 = np.maximum(np.maximum(w1, 0.0) @ W(fp["de2"]), 0.0)
    alpha = float((r2 @ W(fp["de3"])).mean() + bv(fp["de3"]).mean())

    wts = {}
    for i in range(3):
        wts[f"l1w{i}"] = W(ms["ext"][i]["l1"])
        wts[f"l1b{i}"] = _bvec(bv(ms["ext"][i]["l1"]))
        wts[f"l2w{i}"] = W(ms["ext"][i]["l2"])
        wts[f"l2b{i}"] = _bvec(bv(ms["ext"][i]["l2"]))
    wts["attw"] = W(ms["att"])                      # [768, 3]
    wts["attb_bc"] = np.tile(bv(ms["att"])[None, :], (128, 1))   # [128, 3]
    wts["lpw"], wts["lpb"] = W(fp["lig_proj"]), _bvec(bv(fp["lig_proj"]))
    wts["ppw"], wts["ppb"] = W(fp["poc_proj"]), _bvec(bv(fp["poc_proj"]))
    wts["glw"], wts["glb"] = W(fp["gate_l"]), _bvec(bv(fp["gate_l"]))
    wts["gpw"], wts["gpb"] = W(fp["gate_p"]), _bvec(bv(fp["gate_p"]))
    wts["wq"], wts["bq96"] = W(fp["wq"]), _bvec(bv(fp["wq"]), parts=96)
    wts["wk"], wts["bk96"] = W(fp["wk"]), _bvec(bv(fp["wk"]), parts=96)
    wts["wv"] = W(fp["wv"])
    wts["bv_bc"] = np.tile(bv(fp["wv"])[None, :], (128, 1))      # [128, 256]
    wts["wo"], wts["bo"] = W(fp["wo"]), _bvec(bv(fp["wo"]))
    wts["lnlg"], wts["lnlb"] = _bvec(fp["ln_l"]["g"]), _bvec(fp["ln_l"]["b"])
    wts["lnpg"], wts["lnpb"] = _bvec(fp["ln_p"]["g"]), _bvec(fp["ln_p"]["b"])
    wts["fc1w"], wts["fc1b"] = W(mp["fc1"]), _bvec(bv(mp["fc1"]))
    wts["fc2w"], wts["fc2b"] = W(mp["fc2"]), _bvec(bv(mp["fc2"]))
    wts["fc3w"], wts["fc3b"] = W(mp["fc3"]), _bvec(bv(mp["fc3"]))
    wts["pj2w"], wts["pj2b"] = W(mp["proj2"]), _bvec(bv(mp["proj2"]))
    wts["fc4w"] = W(mp["fc4"])                      # [128, 1]
    wts["fc4b"] = _np(mp["fc4"]["b"]).reshape(1, 1)
    wts["bn1g"], wts["bn1b"] = _bvec(mp["bn1"]["g"]), _bvec(mp["bn1"]["b"])
    wts["bn2g"], wts["bn2b"] = _bvec(mp["bn2"]["g"]), _bvec(mp["bn2"]["b"])
    wts["bn3g"], wts["bn3b"] = _bvec(mp["bn3"]["g"]), _bvec(mp["bn3"]["b"])
    wts["ident"] = np.eye(128, dtype=np.float32)
    wts["ones2d"] = np.ones((128, 128), np.float32)     # bcast / partition-sum
    wts["eps"] = np.full((128, 1), 1e-5, np.float32)
    wts["thr"] = np.tile(np.array([s * s for s in SCALES], np.float32)[None, :],
                         (128, 1))                       # [128, 3]

    def aug(c):
        # rows: [cx, cy, cz, n2, 1] (a-form) ; [-2cx,-2cy,-2cz, 1, n2] (b-form)
        n2 = (c ** 2).sum(-1)
        one = np.ones_like(n2)
        a = np.stack([c[:, 0], c[:, 1], c[:, 2], n2, one], 0)
        b = np.stack([-2 * c[:, 0], -2 * c[:, 1], -2 * c[:, 2], one, n2], 0)
        return _np(a), _np(b)

    augL_a, augL_b = aug(lc)
    augP_a, augP_b = aug(pc)

    full_in = {
        "xl_ext": np.concatenate([lig_x, np.ones((NL, 1), np.float32)], 1),
        "xp_ext": np.concatenate([poc_x, np.ones((NP, 1), np.float32)], 1),
        "augL_a": augL_a, "augL_b": augL_b,
        "augP_a": augP_a, "augP_b": augP_b,
    }
    per_core = []
    gids = np.arange(B)
    for c in range(NC):
        ls = slice(c * LLOC, (c + 1) * LLOC)
        ps = slice(c * PLOC, (c + 1) * PLOC)
        per_core.append({
            "my_augL_a": _np(augL_a[:, ls]), "my_augL_b": _np(augL_b[:, ls]),
            "my_augP_a": _np(augP_a[:, ps]), "my_augP_b": _np(augP_b[:, ps]),
            "my_xl_fm": _np(lig_x[ls].T), "my_xp_fm": _np(poc_x[ps].T),
            "my_ohl": _np(lgid[ls][:, None] == gids[None, :]),
            "my_ohp": _np(pgid[ps][:, None] == gids[None, :]),
        })
    return wts, full_in, per_core, alpha


def _build(wts, full_shapes, alpha):
    nc = bacc.Bacc("TRN2", target_bir_lowering=False, debug=False,
                   num_devices=NC)

    # ---- dram I/O ----
    din = {}
    for name, shp in full_shapes.items():
        din[name] = nc.dram_tensor(name, list(shp), F32, kind="ExternalInput")
    out_d = nc.dram_tensor("out", [B, 1], F32, kind="ExternalOutput")

    # collective bounce buffers
    ag1l_i = nc.dram_tensor("ag1l_i", [LLOC, H], F32)
    ag1l_o = nc.dram_tensor("ag1l_o", [NL, H], F32, addr_space="Shared")
    ag1p_i = nc.dram_tensor("ag1p_i", [PLOC, H], F32)
    ag1p_o = nc.dram_tensor("ag1p_o", [NP, H], F32, addr_space="Shared")
    ag2l_i = nc.dram_tensor("ag2l_i", [H, LLOC], F32)
    ag2l_o = nc.dram_tensor("ag2l_o", [NC * H, LLOC], F32, addr_space="Shared")
    ag2p_i = nc.dram_tensor("ag2p_i", [H, PLOC], F32)
    ag2p_o = nc.dram_tensor("ag2p_o", [NC * H, PLOC], F32, addr_space="Shared")
    ar_i = nc.dram_tensor("ar_i", [B, 2 * H + 2], F32)
    ar_o = nc.dram_tensor("ar_o", [B, 2 * H + 2], F32, addr_space="Shared")

    # weight consts
    wd = {k: nc.inline_tensor(v, name=f"w_{k}") for k, v in wts.items()}

    RG = [list(range(NC))]

    with tile.TileContext(nc) as tc:
        with (
            tc.tile_pool(name="persist", bufs=1) as pp_,
            tc.tile_pool(name="scratch", bufs=2) as sp_,
            tc.tile_pool(name="psA", bufs=1, space="PSUM") as psA,
            tc.tile_pool(name="psB", bufs=3, space="PSUM") as psB,
        ):
            def ptile(name, shape):
                return pp_.tile(shape, F32, tag=name, name=name)

            def stile(shape, tag, bufs=None):
                return sp_.tile(shape, F32, tag=tag, name=tag, bufs=bufs)

            # ---------- load constants ----------
            wsb = {}
            for k, v in wts.items():
                kdim, m = v.shape
                if kdim > 128:
                    cs = kdim // 128
                    t = ptile(f"w_{k}", [128, cs, m])
                    nc.sync.dma_start(
                        t[:], wd[k].ap().rearrange("(c p) m -> p c m", p=128))
                else:
                    t = ptile(f"w_{k}", [kdim, m])
                    nc.sync.dma_start(t[:], wd[k].ap())
                wsb[k] = t
            ident = wsb["ident"]

            # ---------- load inputs ----------
            def load_chunked(name, n, m):
                chs = _chunks(n)
                t = ptile(name, [128, len(chs), m])
                for ci, (off, sz) in enumerate(chs):
                    nc.sync.dma_start(t[:sz, ci, :],
                                      din[name].ap()[off:off + sz, :])
                return t

            xl_ext = load_chunked("xl_ext", NL, H + 1)     # [128, 3, 257]
            xp_ext = load_chunked("xp_ext", NP, H + 1)     # [128, 7, 257]
            sb_in = {}
            for k in ("augL_a", "augL_b", "augP_a", "augP_b", "my_augL_a",
                      "my_augL_b", "my_augP_a", "my_augP_b", "my_xl_fm",
                      "my_xp_fm", "my_ohl", "my_ohp"):
                shp = full_shapes[k]
                if shp[0] > 128:
                    t = ptile(k, [128, shp[0] // 128, shp[1]])
                    nc.sync.dma_start(
                        t[:], din[k].ap().rearrange("(c p) m -> p c m", p=128))
                else:
                    t = ptile(k, list(shp))
                    nc.sync.dma_start(t[:], din[k].ap())
                sb_in[k] = t

            # ---------- helpers ----------
            def fm_linear(out_t, x_t, wkey, bkey, func, nloc, Mout,
                          x_chunks=None):
                """out_t[:, mb, :] = func(W^T x + b); x_t [128, KC, nloc] fm."""
                w_t = wsb[wkey]
                b_t = wsb[bkey] if bkey else None
                KC = w_t.shape[1] if len(w_t.shape) == 3 else 1
                xs = x_chunks if x_chunks is not None else \
                    [x_t[:, i, :] for i in range(KC)]
                for mb, (moff, msz) in enumerate(_chunks(Mout)):
                    ps = psB.tile([128, nloc], F32, tag="ps")
                    for i, x in enumerate(xs):
                        lhs = w_t[:, i, moff:moff + msz] if KC > 1 else \
                            w_t[:, moff:moff + msz]
                        nc.tensor.matmul(ps[:msz, :], lhs, x,
                                         start=(i == 0), stop=(i == len(xs) - 1))
                    bias = b_t[:msz, mb:mb + 1] if b_t is not None else 0.0
                    nc.scalar.activation(out_t[:msz, mb, :], ps[:msz, :],
                                         func, bias=bias)

            def fm_to_tm(dst_tm, src_fm, T, C):
                """src [128, C, T] fm -> dst [T, C*128] tm via PE transpose."""
                for c in range(C):
                    ps = psB.tile([128, 128], F32, tag="ps")
                    nc.tensor.transpose(ps[:T, :128], src_fm[:, c, :], ident[:])
                    nc.vector.tensor_copy(dst_tm[:, c * 128:(c + 1) * 128],
                                          ps[:T, :128])

            def tm_to_fm(dst_fm, src_tm, T, kchs):
                """src [T, F] tm -> dst [128, len(kchs), T] fm."""
                for ci, (off, kj) in enumerate(kchs):
                    ps = psB.tile([128, 128], F32, tag="ps")
                    nc.tensor.transpose(ps[:kj, :T], src_tm[:, off:off + kj],
                                        ident[:T, :T])
                    nc.vector.tensor_copy(dst_fm[:kj, ci, :], ps[:kj, :T])

            def bcast_row(row_ap, nloc, base=0):
                """[1, nloc] row at partition `base` -> psum [128, nloc]."""
                ps = psB.tile([128, nloc], F32, tag="ps")
                nc.tensor.matmul(ps[:], wsb["ones2d"][base:base + 1, :], row_ap,
                                 start=True, stop=True)
                return ps

            # ---------- multiscale ----------
            def multiscale(pfx, n_full, nloc, aug_a, my_aug_b, x_ext, my_x_fm):
                kchs = _chunks(n_full)
                nch = len(kchs)
                # adjacency for all chunks/scales
                adj = ptile(f"{pfx}_adj", [128, nch, 3, nloc])
                for ci, (off, kj) in enumerate(kchs):
                    psd = psB.tile([128, nloc], F32, tag="ps")
                    nc.tensor.matmul(psd[:kj, :], aug_a[:, off:off + kj],
                                     my_aug_b[:], start=True, stop=True)
                    nc.vector.tensor_tensor(
                        adj[:kj, ci, :, :],
                        psd[:kj, None, :].to_broadcast((kj, 3, nloc)),
                        wsb["thr"][:kj, :, None].to_broadcast((kj, 3, nloc)),
                        ALU.is_le)
                sf = []
                for s in range(3):
                    # neigh (fm) + row-count via ones column of x_ext
                    nb0 = psB.tile([128, nloc], F32, tag="acc0", bufs=1)
                    nb1 = psB.tile([128, nloc], F32, tag="acc1", bufs=1)
                    nbs = psB.tile([1, nloc], F32, tag="acc2", bufs=1)
                    for ci, (off, kj) in enumerate(kchs):
                        st, sp = ci == 0, ci == nch - 1
                        nc.tensor.matmul(nb0[:], x_ext[:kj, ci, 0:128],
                                         adj[:kj, ci, s, :], start=st, stop=sp)
                        nc.tensor.matmul(nb1[:], x_ext[:kj, ci, 128:256],
                                         adj[:kj, ci, s, :], start=st, stop=sp)
                        nc.tensor.matmul(nbs[:], x_ext[:kj, ci, 256:257],
                                         adj[:kj, ci, s, :], start=st, stop=sp)
                    rs = stile([1, nloc], f"rs_{nloc}")
                    nc.vector.tensor_scalar_add(rs[:], nbs[:], 1e-8)
                    nc.vector.reciprocal(rs[:], rs[:])
                    rb = bcast_row(rs[:], nloc)
                    rbs = stile([128, nloc], f"rbs_{nloc}")
                    nc.vector.tensor_copy(rbs[:], rb[:])
                    neigh = stile([128, 2, nloc], f"neigh_{nloc}")
                    nc.vector.tensor_tensor(neigh[:, 0, :], nb0[:], rbs[:], ALU.mult)
                    nc.vector.tensor_tensor(neigh[:, 1, :], nb1[:], rbs[:], ALU.mult)
                    h1 = stile([128, 2, nloc], f"h1_{nloc}")
                    fm_linear(h1, neigh, f"l1w{s}", f"l1b{s}", AF.Relu, nloc, H)
                    sfs = ptile(f"{pfx}_sf{s}", [128, 2, nloc])
                    fm_linear(sfs, h1, f"l2w{s}", f"l2b{s}", AF.Identity, nloc, H)
                    sf.append(sfs)
                # attention over scales (token-major)
                psa = psB.tile([nloc, 3], F32, tag="ps")
                k = 0
                for s in range(3):
                    for hb in range(2):
                        nc.tensor.matmul(psa[:], sf[s][:, hb, :],
                                         wsb["attw"][:, k, :],
                                         start=(k == 0), stop=(k == 5))
                        k += 1
                att_tm = stile([nloc, 3], f"atttm_{nloc}")
                nc.vector.tensor_tensor(att_tm[:], psa[:],
                                        wsb["attb_bc"][:nloc, :], ALU.add)
                ea = stile([nloc, 3], f"ea_{nloc}")
                ssum = stile([nloc, 1], f"ssum_{nloc}")
                nc.scalar.activation(ea[:], att_tm[:], AF.Exp, accum_out=ssum[:])
                nc.vector.reciprocal(ssum[:], ssum[:])
                nc.vector.tensor_scalar_mul(ea[:], ea[:], ssum[:])
                # per-scale column transpose -> [1, nloc] rows at partition 0
                att_rows = stile([1, 3, nloc], f"attr_{nloc}")
                for s in range(3):
                    pst = psB.tile([128, 128], F32, tag="ps")
                    nc.tensor.transpose(pst[:1, :nloc], ea[:, s:s + 1],
                                        ident[:nloc, :nloc])
                    nc.vector.tensor_copy(att_rows[:, s, :], pst[:1, :nloc])
                # fused = sum_s sf_s * att_s ; v = x + fused
                v_fm = ptile(f"{pfx}_vfm", [128, 2, nloc])
                for s in range(3):
                    ab = bcast_row(att_rows[:, s, :], nloc)
                    for hb in range(2):
                        if s == 0:
                            nc.vector.tensor_tensor(v_fm[:, hb, :],
                                                    sf[s][:, hb, :], ab[:],
                                                    ALU.mult)
                        else:
                            t = stile([128, nloc], f"fus_{nloc}")
                            nc.vector.tensor_tensor(t[:], sf[s][:, hb, :],
                                                    ab[:], ALU.mult)
                            nc.vector.tensor_tensor(v_fm[:, hb, :],
                                                    v_fm[:, hb, :], t[:],
                                                    ALU.add)
                for hb in range(2):
                    nc.vector.tensor_tensor(v_fm[:, hb, :], v_fm[:, hb, :],
                                            my_x_fm[:, hb, :], ALU.add)
                return v_fm

            vl_fm = multiscale("l", NL, LLOC, sb_in["augL_a"],
                               sb_in["my_augL_b"], xl_ext, sb_in["my_xl_fm"])
            vp_fm = multiscale("p", NP, PLOC, sb_in["augP_a"],
                               sb_in["my_augP_b"], xp_ext, sb_in["my_xp_fm"])

            # ---------- projections + AG1 ----------
            lp_fm = ptile("lp_fm", [128, 2, LLOC])
            fm_linear(lp_fm, vl_fm, "lpw", "lpb", AF.Identity, LLOC, H)
            pp_fm = ptile("pp_fm", [128, 2, PLOC])
            fm_linear(pp_fm, vp_fm, "ppw", "ppb", AF.Identity, PLOC, H)

            lp_tm = ptile("lp_tm", [LLOC, H])
            fm_to_tm(lp_tm, lp_fm, LLOC, 2)
            pp_tm = ptile("pp_tm", [PLOC, H])
            fm_to_tm(pp_tm, pp_fm, PLOC, 2)
            nc.sync.dma_start(ag1l_i.ap(), lp_tm[:])
            nc.sync.dma_start(ag1p_i.ap(), pp_tm[:])
            nc.gpsimd.collective_compute(
                "AllGather", ALU.bypass, replica_groups=RG,
                ins=[ag1l_i.ap().opt()], outs=[ag1l_o.ap().opt()])
            nc.gpsimd.collective_compute(
                "AllGather", ALU.bypass, replica_groups=RG,
                ins=[ag1p_i.ap().opt()], outs=[ag1p_o.ap().opt()])
            lchs, pchs = _chunks(NL), _chunks(NP)
            lp_full = ptile("lp_full", [128, len(lchs), H])
            for ci, (off, sz) in enumerate(lchs):
                nc.sync.dma_start(lp_full[:sz, ci, :],
                                  ag1l_o.ap()[off:off + sz, :])
            pp_full = ptile("pp_full", [128, len(pchs), H])
            for ci, (off, sz) in enumerate(pchs):
                nc.sync.dma_start(pp_full[:sz, ci, :],
                                  ag1p_o.ap()[off:off + sz, :])

            # ---------- distance softmax + aggregation + gate + enh ----------
            def fuse_side(pfx, nq, nk, my_aug_a, aug_b_full, opp_full,
                          opp_kchs, q_proj_fm, gw, gb):
                # d2 [nq, nk] -> wl = softmax(-alpha * sqrt(max(d2,0)))
                psd = psA.tile([nq, nk], F32, tag="big")
                for (off, w) in _chunks(nk, 512):
                    nc.tensor.matmul(psd[:, off:off + w], my_aug_a[:],
                                     aug_b_full[:, off:off + w],
                                     start=True, stop=True)
                dpos = stile([nq, nk], "sm", bufs=3)
                nc.vector.tensor_scalar_max(dpos[:], psd[:], 0.0)
                dd = stile([nq, nk], "sm", bufs=3)
                nc.scalar.activation(dd[:], dpos[:], AF.Sqrt)
                ee = stile([nq, nk], "sm", bufs=3)
                ssum = stile([nq, 1], f"ss_{pfx}")
                nc.scalar.activation(ee[:], dd[:], AF.Exp, scale=float(-alpha),
                                     accum_out=ssum[:])
                nc.vector.reciprocal(ssum[:], ssum[:])
                wl = stile([nq, nk], "sm", bufs=3)
                nc.vector.tensor_scalar_mul(wl[:], ee[:], ssum[:])
                # transpose wl -> [nk, nq] chunks, then agg = wl @ opp_full (fm)
                kchs = _chunks(nk)
                wlT = stile([128, len(kchs), nq], "at_att")
                tm_to_fm(wlT, wl, nq, kchs)
                agg = stile([128, 2, nq], f"agg_{pfx}")
                for hb in range(2):
                    ps = psB.tile([128, nq], F32, tag="ps")
                    for ci, (off, kj) in enumerate(kchs):
                        nc.tensor.matmul(
                            ps[:], opp_full[:kj, ci, hb * 128:(hb + 1) * 128],
                            wlT[:kj, ci, :],
                            start=(ci == 0), stop=(ci == len(kchs) - 1))
                    nc.vector.tensor_copy(agg[:, hb, :], ps[:])
                # gate: sigmoid(W^T [proj; agg] + b)
                gate = stile([128, 2, nq], f"gate_{pfx}")
                xs = [q_proj_fm[:, 0, :], q_proj_fm[:, 1, :],
                      agg[:, 0, :], agg[:, 1, :]]
                fm_linear(gate, None, gw, gb, AF.Sigmoid, nq, H, x_chunks=xs)
                # enh = agg + gate*(proj - agg)
                enh = ptile(f"enh_{pfx}", [128, 2, nq])
                for hb in range(2):
                    d = stile([128, nq], f"gd_{pfx}")
                    nc.vector.tensor_tensor(d[:], q_proj_fm[:, hb, :],
                                            agg[:, hb, :], ALU.subtract)
                    nc.vector.tensor_tensor(d[:], gate[:, hb, :], d[:], ALU.mult)
                    nc.vector.tensor_tensor(enh[:, hb, :], agg[:, hb, :], d[:],
                                            ALU.add)
                return enh

            le_fm = fuse_side("l", LLOC, NP, sb_in["my_augL_a"],
                              sb_in["augP_b"], pp_full, pchs, lp_fm,
                              "glw", "glb")
            pe_fm = fuse_side("p", PLOC, NL, sb_in["my_augP_a"],
                              sb_in["augL_b"], lp_full, lchs, pp_fm,
                              "gpw", "gpb")

            # ---------- AG2 (enhanced features, feature-major) ----------
            nc.sync.dma_start(ag2l_i.ap().rearrange("(c p) t -> p c t", p=128),
                              le_fm[:])
            nc.sync.dma_start(ag2p_i.ap().rearrange("(c p) t -> p c t", p=128),
                              pe_fm[:])
            nc.gpsimd.collective_compute(
                "AllGather", ALU.bypass, replica_groups=RG,
                ins=[ag2l_i.ap().opt()], outs=[ag2l_o.ap().opt()])
            nc.gpsimd.collective_compute(
                "AllGather", ALU.bypass, replica_groups=RG,
                ins=[ag2p_i.ap().opt()], outs=[ag2p_o.ap().opt()])
            le_full = ptile("le_full", [128, 2, NL])
            pe_full = ptile("pe_full", [128, 2, NP])
            for g in range(NC):
                for hb in range(2):
                    nc.sync.dma_start(
                        le_full[:, hb, g * LLOC:(g + 1) * LLOC],
                        ag2l_o.ap()[g * H + hb * 128: g * H + hb * 128 + 128, :])
                    nc.sync.dma_start(
                        pe_full[:, hb, g * PLOC:(g + 1) * PLOC],
                        ag2p_o.ap()[g * H + hb * 128: g * H + hb * 128 + 128, :])

            # ---------- cross attention + LN + partial readout ----------
            # q/k live in 96-row blocks so per-head 32-row slices sit at
            # base partitions {0, 32, 64} (matmul constraint).
            QBLK = [(0, 96), (96, 96), (192, 64)]

            def attn_side(pfx, nq, nk, q_src, kv_full, lng, lnb, oh_t):
                q_fm = stile([96, 3, nq], f"q_{pfx}", bufs=1)
                for b, (moff, msz) in enumerate(QBLK):
                    ps = psB.tile([128, nq], F32, tag="ps")
                    for i in range(2):
                        nc.tensor.matmul(ps[:msz, :],
                                         wsb["wq"][:, i, moff:moff + msz],
                                         q_src[:, i, :],
                                         start=(i == 0), stop=(i == 1))
                    nc.scalar.activation(q_fm[:msz, b, :], ps[:msz, :],
                                         AF.Identity,
                                         bias=wsb["bq96"][:msz, b:b + 1])
                k_fm = stile([96, 3, nk], "k_att", bufs=1)
                for b, (moff, msz) in enumerate(QBLK):
                    for (off, w) in _chunks(nk, 512):
                        ps = psB.tile([128, 512], F32, tag="ps")
                        for i in range(2):
                            nc.tensor.matmul(
                                ps[:msz, :w], wsb["wk"][:, i, moff:moff + msz],
                                kv_full[:, i, off:off + w],
                                start=(i == 0), stop=(i == 1))
                        nc.scalar.activation(k_fm[:msz, b, off:off + w],
                                             ps[:msz, :w], AF.Identity,
                                             bias=wsb["bk96"][:msz, b:b + 1])
                tchs = _chunks(nk)
                v_tm = stile([128, len(tchs), H], "v_att", bufs=1)
                for ci, (off, sz) in enumerate(tchs):
                    ps = psB.tile([128, H], F32, tag="ps")
                    for i in range(2):
                        nc.tensor.matmul(ps[:sz, :], kv_full[:, i, off:off + sz],
                                         wsb["wv"][:, i, :],
                                         start=(i == 0), stop=(i == 1))
                    nc.vector.tensor_tensor(v_tm[:sz, ci, :], ps[:sz, :],
                                            wsb["bv_bc"][:sz, :], ALU.add)
                o_fm = stile([128, 2, nq], f"o_{pfx}", bufs=1)
                for h in range(HEADS):
                    blk, br = divmod(h, 3)
                    rows = slice(br * DH, (br + 1) * DH)
                    pss = psA.tile([nq, nk], F32, tag="big")
                    for (off, w) in _chunks(nk, 512):
                        nc.tensor.matmul(pss[:, off:off + w],
                                         q_fm[rows, blk, :],
                                         k_fm[rows, blk, off:off + w],
                                         start=True, stop=True)
                    ee = stile([nq, nk], "sm", bufs=3)
                    ssum = stile([nq, 1], f"as_{pfx}")
                    nc.scalar.activation(ee[:], pss[:], AF.Exp,
                                         scale=float(1.0 / np.sqrt(DH)),
                                         accum_out=ssum[:])
                    nc.vector.reciprocal(ssum[:], ssum[:])
                    nc.vector.tensor_scalar_mul(ee[:], ee[:], ssum[:])
                    at = stile([128, len(tchs), nq], "at_att")
                    tm_to_fm(at, ee, nq, tchs)
                    pso = psB.tile([DH, nq], F32, tag="ps")
                    for ci, (off, kj) in enumerate(tchs):
                        nc.tensor.matmul(pso[:],
                                         v_tm[:kj, ci, h * DH:(h + 1) * DH],
                                         at[:kj, ci, :],
                                         start=(ci == 0), stop=(ci == len(tchs) - 1))
                    ohb, ohr = divmod(h, 4)
                    nc.vector.tensor_copy(
                        o_fm[ohr * DH:(ohr + 1) * DH, ohb, :], pso[:])
                ao_fm = stile([128, 2, nq], f"ao_{pfx}")
                fm_linear(ao_fm, o_fm, "wo", "bo", AF.Identity, nq, H)
                # residual + layernorm (over features = partitions)
                li = stile([128, 2, nq], f"li_{pfx}")
                for hb in range(2):
                    nc.vector.tensor_tensor(li[:, hb, :], q_src[:, hb, :],
                                            ao_fm[:, hb, :], ALU.add)
                pss1 = psB.tile([1, nq], F32, tag="ps")
                for hb in range(2):
                    nc.tensor.matmul(pss1[:], wsb["ones2d"][:, 0:1], li[:, hb, :],
                                     start=(hb == 0), stop=(hb == 1))
                mu = stile([1, nq], f"mu_{pfx}")
                nc.scalar.activation(mu[:], pss1[:], AF.Copy, scale=1.0 / H)
                sq = stile([128, 2, nq], f"sq_{pfx}")
                for hb in range(2):
                    nc.vector.tensor_tensor(sq[:, hb, :], li[:, hb, :],
                                            li[:, hb, :], ALU.mult)
                pss2 = psB.tile([1, nq], F32, tag="ps")
                for hb in range(2):
                    nc.tensor.matmul(pss2[:], wsb["ones2d"][:, 0:1], sq[:, hb, :],
                                     start=(hb == 0), stop=(hb == 1))
                var = stile([1, nq], f"var_{pfx}")
                mu2 = stile([1, nq], f"mu2_{pfx}")
                nc.scalar.activation(mu2[:], mu[:], AF.Square)
                nc.vector.tensor_scalar_mul(var[:], pss2[:], 1.0 / H)
                nc.vector.tensor_tensor(var[:], var[:], mu2[:], ALU.subtract)
                nc.scalar.activation(var[:], var[:], AF.Sqrt, bias=wsb["eps"][:1, :])
                nc.vector.reciprocal(var[:], var[:])
                mub = bcast_row(mu[:], nq)
                rsb = bcast_row(var[:], nq)
                lf = stile([128, 2, nq], f"lf_{pfx}")
                for hb in range(2):
                    nc.vector.tensor_tensor(lf[:, hb, :], li[:, hb, :], mub[:],
                                            ALU.subtract)
                    nc.vector.tensor_tensor(lf[:, hb, :], lf[:, hb, :], rsb[:],
                                            ALU.mult)
                    nc.vector.tensor_scalar(lf[:, hb, :], lf[:, hb, :],
                                            wsb[lng][:, hb:hb + 1],
                                            wsb[lnb][:, hb:hb + 1],
                                            ALU.mult, ALU.add)
                # tm + ones col, then one-hot segment matmul
                lf_ext = stile([nq, H + 1], f"lfe_{pfx}")
                fm_to_tm(lf_ext, lf, nq, 2)
                nc.vector.memset(lf_ext[:, H:H + 1], 1.0)
                psg = psB.tile([B, H + 1], F32, tag="ps")
                nc.tensor.matmul(psg[:], oh_t[:], lf_ext[:],
                                 start=True, stop=True)
                seg = stile([B, H + 1], f"seg_{pfx}")
                nc.vector.tensor_copy(seg[:], psg[:])
                return seg

            seg_l = attn_side("l", LLOC, NP, le_fm, pe_full, "lnlg", "lnlb",
                              sb_in["my_ohl"])
            seg_p = attn_side("p", PLOC, NL, pe_fm, le_full, "lnpg", "lnpb",
                              sb_in["my_ohp"])

            # ---------- AllReduce readout ----------
            nc.sync.dma_start(ar_i.ap()[:, 0:H + 1], seg_l[:])
            nc.sync.dma_start(ar_i.ap()[:, H + 1:2 * H + 2], seg_p[:])
            nc.gpsimd.collective_compute(
                "AllReduce", ALU.add, replica_groups=RG,
                ins=[ar_i.ap().opt()], outs=[ar_o.ap().opt()])
            tot = ptile("tot", [B, 2 * H + 2])
            nc.sync.dma_start(tot[:], ar_o.ap())

            # ---------- decoder (replicated) ----------
            x_tm = ptile("x_tm", [B, 2 * H])
            cnt = stile([B, 1], "cnt")
            nc.vector.tensor_scalar_max(cnt[:], tot[:, H:H + 1], 1.0)
            nc.vector.reciprocal(cnt[:], cnt[:])
            nc.vector.tensor_scalar_mul(x_tm[:, 0:H], tot[:, 0:H], cnt[:])
            cnt2 = stile([B, 1], "cnt2")
            nc.vector.tensor_scalar_max(cnt2[:], tot[:, 2 * H + 1:2 * H + 2], 1.0)
            nc.vector.reciprocal(cnt2[:], cnt2[:])
            nc.vector.tensor_scalar_mul(x_tm[:, H:2 * H],
                                        tot[:, H + 1:2 * H + 1], cnt2[:])

            def to_fm(src_tm, Fdim, name):
                t = ptile(name, [128, Fdim // 128, B])
                tm_to_fm(t, src_tm, B, _chunks(Fdim))
                return t

            x_fm = to_fm(x_tm, 2 * H, "x_fm")

            def bn_leaky(dst, src_psum_sb, mb, gkey, bkey):
                """dst[:,mb,:] = leaky(bn(src)); src is [128, B] sbuf."""
                s1 = stile([128, 1], "bn_s1")
                nc.vector.tensor_reduce(s1[:], src_psum_sb[:], mybir.AxisListType.X,
                                        ALU.add)
                sq = stile([128, B], "bn_sq")
                nc.vector.tensor_tensor(sq[:], src_psum_sb[:], src_psum_sb[:],
                                        ALU.mult)
                s2 = stile([128, 1], "bn_s2")
                nc.vector.tensor_reduce(s2[:], sq[:], mybir.AxisListType.X, ALU.add)
                muv = stile([128, 1], "bn_mu")
                nc.scalar.activation(muv[:], s1[:], AF.Copy, scale=1.0 / B)
                mu2 = stile([128, 1], "bn_mu2")
                nc.scalar.activation(mu2[:], s1[:], AF.Square, scale=1.0 / B)
                varv = stile([128, 1], "bn_var")
                nc.vector.tensor_scalar_mul(varv[:], s2[:], 1.0 / B)
                nc.vector.tensor_tensor(varv[:], varv[:], mu2[:], ALU.subtract)
                nc.scalar.activation(varv[:], varv[:], AF.Sqrt, bias=wsb["eps"][:, 0:1])
                nc.vector.reciprocal(varv[:], varv[:])
                bnv = stile([128, B], "bn_out")
                nc.vector.tensor_scalar(bnv[:], src_psum_sb[:], muv[:], varv[:],
                                        ALU.subtract, ALU.mult)
                nc.vector.tensor_scalar(bnv[:], bnv[:],
                                        wsb[gkey][:, mb:mb + 1],
                                        wsb[bkey][:, mb:mb + 1],
                                        ALU.mult, ALU.add)
                ab = stile([128, B], "bn_abs")
                nc.scalar.activation(ab[:], bnv[:], AF.Abs, scale=0.45)
                nc.vector.scalar_tensor_tensor(dst[:], bnv[:], 0.55, ab[:],
                                               ALU.mult, ALU.add)

            def fc_layer(x_in, KC, Mout, wkey, bkey, gkey, bnbkey, res_from,
                         name):
                out_t = ptile(name, [128, Mout // 128, B])
                for mb, (moff, msz) in enumerate(_chunks(Mout)):
                    ps = psB.tile([128, B], F32, tag="ps")
                    for i in range(KC):
                        nc.tensor.matmul(ps[:msz, :],
                                         wsb[wkey][:, i, moff:moff + msz],
                                         x_in[:, i, :],
                                         start=(i == 0), stop=(i == KC - 1))
                    pre = stile([128, B], "fc_pre")
                    nc.scalar.activation(pre[:msz, :], ps[:msz, :], AF.Identity,
                                         bias=wsb[bkey][:msz, mb:mb + 1])
                    lk = stile([128, B], "fc_lk")
                    bn_leaky(lk, pre, mb, gkey, bnbkey)
                    if res_from is not None:
                        nc.vector.scalar_tensor_tensor(
                            out_t[:, mb, :], res_from[:, mb, :], 0.1, lk[:],
                            ALU.mult, ALU.add)
                    else:
                        nc.vector.tensor_copy(out_t[:, mb, :], lk[:])
                return out_t

            h1 = fc_layer(x_fm, 4, MLP_HID, "fc1w", "fc1b", "bn1g", "bn1b",
                          x_fm, "h1_fm")
            h2 = fc_layer(h1, 4, MLP_HID, "fc2w", "fc2b", "bn2g", "bn2b",
                          h1, "h2_fm")
            h3pre = fc_layer(h2, 4, MLP_OUT, "fc3w", "fc3b", "bn3g", "bn3b",
                             None, "h3p_fm")
            # proj2 residual: h3 = h3pre + 0.1 * (W_pj2^T h2 + b_pj2)
            psj = psB.tile([128, B], F32, tag="ps")
            for i in range(4):
                nc.tensor.matmul(psj[:], wsb["pj2w"][:, i, :], h2[:, i, :],
                                 start=(i == 0), stop=(i == 3))
            pj = stile([128, B], "pj_out")
            nc.scalar.activation(pj[:], psj[:], AF.Identity,
                                 bias=wsb["pj2b"][:, 0:1])
            h3 = ptile("h3_fm", [128, 1, B])
            nc.vector.scalar_tensor_tensor(h3[:, 0, :], pj[:], 0.1,
                                           h3pre[:, 0, :], ALU.mult, ALU.add)
            psf = psB.tile([1, B], F32, tag="ps")
            nc.tensor.matmul(psf[:], wsb["fc4w"][:, 0:1], h3[:, 0, :],
                             start=True, stop=True)
            yout = ptile("yout", [1, B])
            nc.scalar.activation(yout[:], psf[:], AF.Identity,
                                 bias=wsb["fc4b"][:, 0:1])
            nc.sync.dma_start(out_d.ap().rearrange("a b -> b a"), yout[:])

    nc.compile()
    return nc


def kernel(**inputs) -> np.ndarray:
    wts, full_in, per_core, alpha = _prep_host(inputs)
    full_shapes = {k: v.shape for k, v in full_in.items()}
    full_shapes.update({k: v.shape for k, v in per_core[0].items()})
    nc = _build(wts, full_shapes, alpha)
    in_maps = [{**full_in, **pc} for pc in per_core]
    res = run_bass_kernel_spmd(nc, in_maps, core_ids=list(range(NC)))
    return np.asarray(res.results[0]["out"], dtype=np.float32)


if __name__ == "__main__":
    import jax
    jax.config.update("jax_platforms", "cpu")
    import reference as R
    inp = R.setup_inputs()
    ref = np.array(R.reference(**inp))
    act = kernel(**inp)
    rel = np.linalg.norm(act - ref) / np.linalg.norm(ref)
    print("rel fro err:", rel)


# revision 23
# speedup vs baseline: 1.2028x; 1.2028x over previous
"""DrugBAN3D Trainium2 kernel — 8-core SPMD Bass/Tile implementation.

Sharding: ligand rows (320) and pocket rows (800) are split 8 ways
(40 / 100 rows per core). Each core computes its rows through
multiscale -> fusion -> cross-attention -> LN -> partial segment sums,
with one AllGather at each of the two points where full node sets are
needed (projected features for the distance-weighted aggregation;
enhanced features for cross-attention K/V) and one AllReduce for the
readout. The decoder (B=32) is replicated on every core.

Key structural fact used (exact, not approximate): the distance-encoder
MLP de3(relu(de2(relu(de1(D))))) has all-zero biases, and D >= 0, so
mean_H(enc) == alpha * D for a scalar alpha = mean(W3^T relu(W2^T relu(w1))).
The [Nl,Np,H] encoding is never materialized.

Layouts: "fm" = feature-major [feat(part), tok(free)] (natural for PE
contractions), "tm" = token-major. d^2 matrices come from K=5 matmuls over
host-prepared augmented coordinate vectors. q/k live in 96-row blocks so
per-head 32-row slices sit at legal base partitions; the ligand side packs
3 heads per scores/transpose/AV matmul via a block-diagonal q.
"""

import sys

for _p in ("/opt/trn_rl_repo", "/root/.axon_site/_ro/trn_rl_repo"):
    if _p not in sys.path:
        sys.path.append(_p)

import numpy as np

import concourse.bass as bass
import concourse.mybir as mybir
import concourse.tile as tile
from concourse import bacc
from concourse.bass_utils import run_bass_kernel_spmd

F32 = mybir.dt.float32
AF = mybir.ActivationFunctionType
ALU = mybir.AluOpType

NC = 8
NL, NP, H, B = 320, 800, 256, 32
LLOC, PLOC = NL // NC, NP // NC     # 40, 100
HEADS, DH = 8, 32
SCALES = (2.0, 5.0, 8.0)
MLP_HID, MLP_OUT = 512, 128


def _np(x):
    return np.ascontiguousarray(np.asarray(x, dtype=np.float32))


def _chunks(n, c=128):
    out = []
    off = 0
    while off < n:
        out.append((off, min(c, n - off)))
        off += c
    return out


def _bvec(b, parts=128):
    b = _np(b)
    m = b.shape[0]
    nch = (m + parts - 1) // parts
    out = np.zeros((parts, nch), np.float32)
    for c in range(nch):
        seg = b[c * parts:(c + 1) * parts]
        out[: seg.shape[0], c] = seg
    return out


def _prep_host(inputs):
    """Host-side preprocessing: packed weights + per-core inputs + alpha."""
    ms, fp, mp = inputs["ms_params"], inputs["fus_params"], inputs["mlp_params"]
    lig_x, poc_x = _np(inputs["lig_x"]), _np(inputs["poc_x"])
    lc, pc = _np(inputs["lig_coords"]), _np(inputs["poc_coords"])
    lgid, pgid = np.asarray(inputs["lig_gid"]), np.asarray(inputs["poc_gid"])

    def W(p):
        return _np(p["w"])

    def bv(p):
        return _np(p["b"])

    w1 = W(fp["de1"])[0]
    r2 = np.maximum(np.maximum(w1, 0.0) @ W(fp["de2"]), 0.0)
    alpha = float((r2 @ W(fp["de3"])).mean() + bv(fp["de3"]).mean())

    wts = {}
    for i in range(3):
        wts[f"l1w{i}"] = W(ms["ext"][i]["l1"])
        wts[f"l1b{i}"] = _bvec(bv(ms["ext"][i]["l1"]))
        wts[f"l2w{i}"] = W(ms["ext"][i]["l2"])
        wts[f"l2b{i}"] = _bvec(bv(ms["ext"][i]["l2"]))
    wts["attw"] = W(ms["att"])
    wts["attb_bc"] = np.tile(bv(ms["att"])[None, :], (128, 1))
    wts["lpw"], wts["lpb"] = W(fp["lig_proj"]), _bvec(bv(fp["lig_proj"]))
    wts["ppw"], wts["ppb"] = W(fp["poc_proj"]), _bvec(bv(fp["poc_proj"]))
    wts["glw"], wts["glb"] = W(fp["gate_l"]), _bvec(bv(fp["gate_l"]))
    wts["gpw"], wts["gpb"] = W(fp["gate_p"]), _bvec(bv(fp["gate_p"]))
    wts["wq"], wts["bq96"] = W(fp["wq"]), _bvec(bv(fp["wq"]), parts=96)
    wts["wk"], wts["bk96"] = W(fp["wk"]), _bvec(bv(fp["wk"]), parts=96)
    wts["wv"] = W(fp["wv"])
    wts["bv_bc"] = np.tile(bv(fp["wv"])[None, :], (128, 1))
    wts["wo"], wts["bo"] = W(fp["wo"]), _bvec(bv(fp["wo"]))
    wts["lnlg"], wts["lnlb"] = _bvec(fp["ln_l"]["g"]), _bvec(fp["ln_l"]["b"])
    wts["lnpg"], wts["lnpb"] = _bvec(fp["ln_p"]["g"]), _bvec(fp["ln_p"]["b"])
    wts["fc1w"], wts["fc1b"] = W(mp["fc1"]), _bvec(bv(mp["fc1"]))
    wts["fc2w"], wts["fc2b"] = W(mp["fc2"]), _bvec(bv(mp["fc2"]))
    wts["fc3w"], wts["fc3b"] = W(mp["fc3"]), _bvec(bv(mp["fc3"]))
    wts["pj2w"], wts["pj2b"] = W(mp["proj2"]), _bvec(bv(mp["proj2"]))
    wts["fc4w"] = W(mp["fc4"])
    wts["fc4b"] = _np(mp["fc4"]["b"]).reshape(1, 1)
    wts["bn1g"], wts["bn1b"] = _bvec(mp["bn1"]["g"]), _bvec(mp["bn1"]["b"])
    wts["bn2g"], wts["bn2b"] = _bvec(mp["bn2"]["g"]), _bvec(mp["bn2"]["b"])
    wts["bn3g"], wts["bn3b"] = _bvec(mp["bn3"]["g"]), _bvec(mp["bn3"]["b"])
    wts["ident"] = np.eye(128, dtype=np.float32)
    wts["ones2d"] = np.ones((128, 128), np.float32)
    wts["eps"] = np.full((128, 1), 1e-5, np.float32)
    wts["thr"] = np.tile(np.array([s * s for s in SCALES], np.float32)[None, :],
                         (128, 1))

    # pack all weights into one [128, F] constant (one DMA at runtime)
    specs = {}
    cols = []
    off = 0
    for k, v in wts.items():
        kdim, m = v.shape
        if kdim > 128:
            c = kdim // 128
            arr = v.reshape(c, 128, m).transpose(1, 0, 2).reshape(128, c * m)
        else:
            arr = np.zeros((128, m), np.float32)
            arr[:kdim] = v
        specs[k] = (off, kdim, m)
        off += arr.shape[1]
        cols.append(arr)
    wbig = np.ascontiguousarray(np.concatenate(cols, axis=1))

    def aug(c):
        n2 = (c ** 2).sum(-1)
        one = np.ones_like(n2)
        a = np.stack([c[:, 0], c[:, 1], c[:, 2], n2, one], 0)
        b = np.stack([-2 * c[:, 0], -2 * c[:, 1], -2 * c[:, 2], one, n2], 0)
        return _np(a), _np(b)

    augL_a, augL_b = aug(lc)
    augP_a, augP_b = aug(pc)

    full_in = {
        "xl_ext": np.concatenate([lig_x, np.ones((NL, 1), np.float32)], 1),
        "xp_ext": np.concatenate([poc_x, np.ones((NP, 1), np.float32)], 1),
        "augL_a": augL_a, "augL_b": augL_b,
        "augP_a": augP_a, "augP_b": augP_b,
    }
    per_core = []
    gids = np.arange(B)
    for c in range(NC):
        ls = slice(c * LLOC, (c + 1) * LLOC)
        ps = slice(c * PLOC, (c + 1) * PLOC)
        per_core.append({
            "my_augL_a": _np(augL_a[:, ls]), "my_augL_b": _np(augL_b[:, ls]),
            "my_augP_a": _np(augP_a[:, ps]), "my_augP_b": _np(augP_b[:, ps]),
            "my_xl_fm": _np(lig_x[ls].T), "my_xp_fm": _np(poc_x[ps].T),
            "my_ohl": _np(lgid[ls][:, None] == gids[None, :]),
            "my_ohp": _np(pgid[ps][:, None] == gids[None, :]),
        })
    return (wbig, specs), full_in, per_core, alpha


def _build(wpack, full_shapes, alpha):
    wbig_np, specs = wpack
    nc = bacc.Bacc("TRN2", target_bir_lowering=False, debug=False,
                   num_devices=NC)

    din = {}
    for name, shp in full_shapes.items():
        din[name] = nc.dram_tensor(name, list(shp), F32, kind="ExternalInput")
    out_d = nc.dram_tensor("out", [B, 1], F32, kind="ExternalOutput")

    # collective bounce buffers (one AG per sync point, lig+poc fused)
    AGR = LLOC + PLOC                                   # 140
    ag1_i = nc.dram_tensor("ag1_i", [AGR, H], F32)
    ag1_o = nc.dram_tensor("ag1_o", [NC * AGR, H], F32, addr_space="Shared")
    ag2_i = nc.dram_tensor("ag2_i", [H, AGR], F32)
    ag2_o = nc.dram_tensor("ag2_o", [NC * H, AGR], F32, addr_space="Shared")
    ar_i = nc.dram_tensor("ar_i", [B, 2 * H + 2], F32)
    ar_o = nc.dram_tensor("ar_o", [B, 2 * H + 2], F32, addr_space="Shared")

    wbig_d = nc.inline_tensor(wbig_np, name="wbig")
    RG = [list(range(NC))]

    with tile.TileContext(nc) as tc:
        with (
            tc.tile_pool(name="persist", bufs=1) as pp_,
            tc.tile_pool(name="scratch", bufs=2) as sp_,
            tc.tile_pool(name="psA", bufs=1, space="PSUM") as psA,
            tc.tile_pool(name="psB", bufs=3, space="PSUM") as psB,
        ):
            def ptile(name, shape):
                return pp_.tile(shape, F32, tag=name, name=name)

            def stile(shape, tag, bufs=None):
                return sp_.tile(shape, F32, tag=tag, name=tag, bufs=bufs)

            # ---------- constants (one DMA) ----------
            wbig_t = ptile("wbig_t", [128, wbig_np.shape[1]])
            nc.sync.dma_start(wbig_t[:], wbig_d.ap())

            def wap(k):
                off, kdim, m = specs[k]
                if kdim > 128:
                    c = kdim // 128
                    return wbig_t[:, off:off + c * m].rearrange(
                        "p (c m) -> p c m", m=m)
                return wbig_t[:kdim, off:off + m]

            wsb = {k: wap(k) for k in specs}
            ident = wsb["ident"]

            # ---------- inputs ----------
            def load_chunked(name, n, m):
                chs = _chunks(n)
                t = ptile(name, [128, len(chs), m])
                for ci, (off, sz) in enumerate(chs):
                    nc.sync.dma_start(t[:sz, ci, :],
                                      din[name].ap()[off:off + sz, :])
                return t

            xl_ext = load_chunked("xl_ext", NL, H + 1)
            xp_ext = load_chunked("xp_ext", NP, H + 1)
            sb_in = {}
            for k in ("augL_a", "augL_b", "augP_a", "augP_b", "my_augL_a",
                      "my_augL_b", "my_augP_a", "my_augP_b", "my_xl_fm",
                      "my_xp_fm", "my_ohl", "my_ohp"):
                shp = full_shapes[k]
                if shp[0] > 128:
                    t = ptile(k, [128, shp[0] // 128, shp[1]])
                    nc.sync.dma_start(
                        t[:], din[k].ap().rearrange("(c p) m -> p c m", p=128))
                else:
                    t = ptile(k, list(shp))
                    nc.sync.dma_start(t[:], din[k].ap())
                sb_in[k] = t

            # ---------- helpers ----------
            def fm_linear(out_t, x_t, wkey, bkey, func, nloc, Mout,
                          x_chunks=None):
                w_t = wsb[wkey]
                b_t = wsb[bkey] if bkey else None
                KC = w_t.shape[1] if len(w_t.shape) == 3 else 1
                xs = x_chunks if x_chunks is not None else \
                    [x_t[:, i, :] for i in range(KC)]
                for mb, (moff, msz) in enumerate(_chunks(Mout)):
                    ps = psB.tile([128, nloc], F32, tag="ps")
                    for i, x in enumerate(xs):
                        lhs = w_t[:, i, moff:moff + msz] if KC > 1 else \
                            w_t[:, moff:moff + msz]
                        nc.tensor.matmul(ps[:msz, :], lhs, x,
                                         start=(i == 0), stop=(i == len(xs) - 1))
                    bias = b_t[:msz, mb:mb + 1] if b_t is not None else 0.0
                    nc.scalar.activation(out_t[:msz, mb, :], ps[:msz, :],
                                         func, bias=bias)

            def fm_to_tm(dst_tm, src_fm, T, C):
                for c in range(C):
                    ps = psB.tile([128, 128], F32, tag="ps")
                    nc.tensor.transpose(ps[:T, :128], src_fm[:, c, :], ident[:])
                    nc.vector.tensor_copy(dst_tm[:, c * 128:(c + 1) * 128],
                                          ps[:T, :128])

            def tm_to_fm(dst_fm, src_tm, T, kchs, width=None):
                for ci, (off, kj) in enumerate(kchs):
                    ps = psB.tile([128, 128], F32, tag="ps")
                    nc.tensor.transpose(ps[:kj, :T], src_tm[:, off:off + kj],
                                        ident[:T, :T])
                    nc.vector.tensor_copy(dst_fm[:kj, ci, :T], ps[:kj, :T])

            def bcast_row(row_ap, nloc, base=0):
                ps = psB.tile([128, nloc], F32, tag="ps")
                nc.tensor.matmul(ps[:], wsb["ones2d"][base:base + 1, :], row_ap,
                                 start=True, stop=True)
                return ps

            # ---------- multiscale (all 3 scales fused per matmul) ----------
            def multiscale(pfx, n_full, nloc, aug_a, my_aug_b, x_ext, my_x_fm):
                kchs = _chunks(n_full)
                nch = len(kchs)
                n3 = 3 * nloc
                adj = ptile(f"{pfx}_adj", [128, nch, 3, nloc])
                for ci, (off, kj) in enumerate(kchs):
                    psd = psB.tile([128, nloc], F32, tag="ps")
                    nc.tensor.matmul(psd[:kj, :], aug_a[:, off:off + kj],
                                     my_aug_b[:], start=True, stop=True)
                    nc.vector.tensor_tensor(
                        adj[:kj, ci, :, :],
                        psd[:kj, None, :].to_broadcast((kj, 3, nloc)),
                        wsb["thr"][:kj, :, None].to_broadcast((kj, 3, nloc)),
                        ALU.is_le)
                nb0 = psB.tile([128, n3], F32, tag="acc0", bufs=1)
                nb1 = psB.tile([128, n3], F32, tag="acc1", bufs=1)
                nbs = psB.tile([1, n3], F32, tag="acc2", bufs=1)
                for ci, (off, kj) in enumerate(kchs):
                    st, sp = ci == 0, ci == nch - 1
                    rhs = adj[:kj, ci, :, :]
                    nc.tensor.matmul(nb0[:], x_ext[:kj, ci, 0:128], rhs,
                                     start=st, stop=sp)
                    nc.tensor.matmul(nb1[:], x_ext[:kj, ci, 128:256], rhs,
                                     start=st, stop=sp)
                    nc.tensor.matmul(nbs[:], x_ext[:kj, ci, 256:257], rhs,
                                     start=st, stop=sp)
                rs = stile([1, n3], f"rs_{nloc}")
                nc.vector.tensor_scalar_add(rs[:], nbs[:], 1e-8)
                nc.vector.reciprocal(rs[:], rs[:])
                rb = bcast_row(rs[:], n3)
                rbs = stile([128, n3], f"rbs_{nloc}")
                nc.vector.tensor_copy(rbs[:], rb[:])
                neigh = stile([128, 2, n3], f"neigh_{nloc}")
                nc.vector.tensor_tensor(neigh[:, 0, :], nb0[:], rbs[:], ALU.mult)
                nc.vector.tensor_tensor(neigh[:, 1, :], nb1[:], rbs[:], ALU.mult)
                sf = []
                for s in range(3):
                    xs = [neigh[:, 0, s * nloc:(s + 1) * nloc],
                          neigh[:, 1, s * nloc:(s + 1) * nloc]]
                    h1 = stile([128, 2, nloc], f"h1_{nloc}")
                    fm_linear(h1, None, f"l1w{s}", f"l1b{s}", AF.Relu, nloc, H,
                              x_chunks=xs)
                    sfs = ptile(f"{pfx}_sf{s}", [128, 2, nloc])
                    fm_linear(sfs, h1, f"l2w{s}", f"l2b{s}", AF.Identity,
                              nloc, H)
                    sf.append(sfs)
                psa = psB.tile([nloc, 3], F32, tag="ps")
                k = 0
                for s in range(3):
                    for hb in range(2):
                        nc.tensor.matmul(psa[:], sf[s][:, hb, :],
                                         wsb["attw"][:, k, :],
                                         start=(k == 0), stop=(k == 5))
                        k += 1
                att_tm = stile([nloc, 3], f"atttm_{nloc}")
                nc.vector.tensor_tensor(att_tm[:], psa[:],
                                        wsb["attb_bc"][:nloc, :], ALU.add)
                ea = stile([nloc, 3], f"ea_{nloc}")
                ssum = stile([nloc, 1], f"ssum_{nloc}")
                nc.scalar.activation(ea[:], att_tm[:], AF.Exp,
                                     accum_out=ssum[:])
                nc.vector.reciprocal(ssum[:], ssum[:])
                nc.vector.tensor_scalar_mul(ea[:], ea[:], ssum[:])
                att_rows = stile([1, 3, nloc], f"attr_{nloc}")
                for s in range(3):
                    pst = psB.tile([128, 128], F32, tag="ps")
                    nc.tensor.transpose(pst[:1, :nloc], ea[:, s:s + 1],
                                        ident[:nloc, :nloc])
                    nc.vector.tensor_copy(att_rows[:, s, :], pst[:1, :nloc])
                v_fm = ptile(f"{pfx}_vfm", [128, 2, nloc])
                for s in range(3):
                    ab = bcast_row(att_rows[:, s, :], nloc)
                    for hb in range(2):
                        if s == 0:
                            nc.vector.tensor_tensor(v_fm[:, hb, :],
                                                    sf[s][:, hb, :], ab[:],
                                                    ALU.mult)
                        else:
                            t = stile([128, nloc], f"fus_{nloc}")
                            nc.vector.tensor_tensor(t[:], sf[s][:, hb, :],
                                                    ab[:], ALU.mult)
                            nc.vector.tensor_tensor(v_fm[:, hb, :],
                                                    v_fm[:, hb, :], t[:],
                                                    ALU.add)
                for hb in range(2):
                    nc.vector.tensor_tensor(v_fm[:, hb, :], v_fm[:, hb, :],
                                            my_x_fm[:, hb, :], ALU.add)
                return v_fm

            with nc.named_scope("ms_lig"):
                vl_fm = multiscale("l", NL, LLOC, sb_in["augL_a"],
                                   sb_in["my_augL_b"], xl_ext,
                                   sb_in["my_xl_fm"])
            with nc.named_scope("ms_poc"):
                vp_fm = multiscale("p", NP, PLOC, sb_in["augP_a"],
                                   sb_in["my_augP_b"], xp_ext,
                                   sb_in["my_xp_fm"])

            # ---------- projections + AG1 (single fused collective) ----------
            with nc.named_scope("proj_ag1"):
                lp_fm = ptile("lp_fm", [128, 2, LLOC])
                fm_linear(lp_fm, vl_fm, "lpw", "lpb", AF.Identity, LLOC, H)
                pp_fm = ptile("pp_fm", [128, 2, PLOC])
                fm_linear(pp_fm, vp_fm, "ppw", "ppb", AF.Identity, PLOC, H)

                lp_tm = ptile("lp_tm", [LLOC, H])
                fm_to_tm(lp_tm, lp_fm, LLOC, 2)
                pp_tm = ptile("pp_tm", [PLOC, H])
                fm_to_tm(pp_tm, pp_fm, PLOC, 2)
                nc.sync.dma_start(ag1_i.ap()[0:LLOC, :], lp_tm[:])
                nc.sync.dma_start(ag1_i.ap()[LLOC:AGR, :], pp_tm[:])
                nc.gpsimd.collective_compute(
                    "AllGather", ALU.bypass, replica_groups=RG,
                    ins=[ag1_i.ap().opt()], outs=[ag1_o.ap().opt()])
                # full projected sets, grouped by source core (g-block layout)
                lp_full = ptile("lp_full", [LLOC, NC, H])
                pp_full = ptile("pp_full", [PLOC, NC, H])
                for g in range(NC):
                    nc.sync.dma_start(
                        lp_full[:, g, :],
                        ag1_o.ap()[g * AGR:g * AGR + LLOC, :])
                    nc.sync.dma_start(
                        pp_full[:, g, :],
                        ag1_o.ap()[g * AGR + LLOC:(g + 1) * AGR, :])

            # ---------- distance softmax + aggregation + gate + enh ----------
            def fuse_side(pfx, nq, nk, nkloc, my_aug_a, aug_b_full, opp_full,
                          q_proj_fm, gw, gb):
                psd = psA.tile([nq, nk], F32, tag="big")
                for (off, w) in _chunks(nk, 512):
                    nc.tensor.matmul(psd[:, off:off + w], my_aug_a[:],
                                     aug_b_full[:, off:off + w],
                                     start=True, stop=True)
                dpos = stile([nq, nk], "sm", bufs=2)
                nc.vector.tensor_scalar_max(dpos[:], psd[:], 0.0)
                dd = stile([nq, nk], "sm", bufs=2)
                nc.scalar.activation(dd[:], dpos[:], AF.Sqrt)
                ee = stile([nq, nk], "sm", bufs=2)
                ssum = stile([nq, 1], f"ss_{pfx}")
                nc.scalar.activation(ee[:], dd[:], AF.Exp, scale=float(-alpha),
                                     accum_out=ssum[:])
                nc.vector.reciprocal(ssum[:], ssum[:])
                wl = stile([nq, nk], "sm", bufs=2)
                nc.vector.tensor_scalar_mul(wl[:], ee[:], ssum[:])
                # transpose wl into source-core blocks matching opp_full
                wlT = stile([128, NC, nq], "at_att")
                for g in range(NC):
                    ps = psB.tile([128, 128], F32, tag="ps")
                    nc.tensor.transpose(ps[:nkloc, :nq],
                                        wl[:, g * nkloc:(g + 1) * nkloc],
                                        ident[:nq, :nq])
                    nc.vector.tensor_copy(wlT[:nkloc, g, :], ps[:nkloc, :nq])
                agg = stile([128, 2, nq], f"agg_{pfx}")
                for hb in range(2):
                    ps = psB.tile([128, nq], F32, tag="ps")
                    for g in range(NC):
                        nc.tensor.matmul(
                            ps[:], opp_full[:, g, hb * 128:(hb + 1) * 128],
                            wlT[:nkloc, g, :],
                            start=(g == 0), stop=(g == NC - 1))
                    nc.vector.tensor_copy(agg[:, hb, :], ps[:])
                gate = stile([128, 2, nq], f"gate_{pfx}")
                xs = [q_proj_fm[:, 0, :], q_proj_fm[:, 1, :],
                      agg[:, 0, :], agg[:, 1, :]]
                fm_linear(gate, None, gw, gb, AF.Sigmoid, nq, H, x_chunks=xs)
                enh = ptile(f"enh_{pfx}", [128, 2, nq])
                for hb in range(2):
                    d = stile([128, nq], f"gd_{pfx}")
                    nc.vector.tensor_tensor(d[:], q_proj_fm[:, hb, :],
                                            agg[:, hb, :], ALU.subtract)
                    nc.vector.tensor_tensor(d[:], gate[:, hb, :], d[:], ALU.mult)
                    nc.vector.tensor_tensor(enh[:, hb, :], agg[:, hb, :], d[:],
                                            ALU.add)
                return enh

            with nc.named_scope("fuse_l"):
                le_fm = fuse_side("l", LLOC, NP, PLOC, sb_in["my_augL_a"],
                                  sb_in["augP_b"], pp_full, lp_fm, "glw", "glb")
            with nc.named_scope("fuse_p"):
                pe_fm = fuse_side("p", PLOC, NL, LLOC, sb_in["my_augP_a"],
                                  sb_in["augL_b"], lp_full, pp_fm, "gpw", "gpb")

            # ---------- AG2 (enhanced features, feature-major, fused) ----------
            with nc.named_scope("ag2"):
                nc.sync.dma_start(
                    ag2_i.ap()[:, 0:LLOC].rearrange("(c p) t -> p c t", p=128),
                    le_fm[:])
                nc.sync.dma_start(
                    ag2_i.ap()[:, LLOC:AGR].rearrange("(c p) t -> p c t", p=128),
                    pe_fm[:])
                nc.gpsimd.collective_compute(
                    "AllGather", ALU.bypass, replica_groups=RG,
                    ins=[ag2_i.ap().opt()], outs=[ag2_o.ap().opt()])
                le_full = ptile("le_full", [128, 2, NL])
                pe_full = ptile("pe_full", [128, 2, NP])
                src = ag2_o.ap().rearrange("(g c p) t -> p c g t", c=2, p=128)
                for hb in range(2):
                    nc.sync.dma_start(
                        le_full[:, hb, :].rearrange("p (g t) -> p g t", g=NC),
                        src[:, hb, :, 0:LLOC])
                    nc.sync.dma_start(
                        pe_full[:, hb, :].rearrange("p (g t) -> p g t", g=NC),
                        src[:, hb, :, LLOC:AGR])

            # ---------- cross attention + LN + partial readout ----------
            QBLK = [(0, 96), (96, 96), (192, 64)]
            # head h lives in 96-block h//3 at rows 32*(h%3)

            def attn_side(pfx, nq, nk, q_src, kv_full, lng, lnb, oh_t, pack):
                q_fm = stile([96, 3, nq], f"q_{pfx}", bufs=1)
                for b, (moff, msz) in enumerate(QBLK):
                    ps = psB.tile([128, nq], F32, tag="ps")
                    for i in range(2):
                        nc.tensor.matmul(ps[:msz, :],
                                         wsb["wq"][:, i, moff:moff + msz],
                                         q_src[:, i, :],
                                         start=(i == 0), stop=(i == 1))
                    nc.scalar.activation(q_fm[:msz, b, :], ps[:msz, :],
                                         AF.Identity,
                                         bias=wsb["bq96"][:msz, b:b + 1])
                k_fm = stile([96, 3, nk], "k_att", bufs=1)
                for b, (moff, msz) in enumerate(QBLK):
                    for (off, w) in _chunks(nk, 512):
                        ps = psB.tile([128, 512], F32, tag="ps")
                        for i in range(2):
                            nc.tensor.matmul(
                                ps[:msz, :w], wsb["wk"][:, i, moff:moff + msz],
                                kv_full[:, i, off:off + w],
                                start=(i == 0), stop=(i == 1))
                        nc.scalar.activation(k_fm[:msz, b, off:off + w],
                                             ps[:msz, :w], AF.Identity,
                                             bias=wsb["bk96"][:msz, b:b + 1])
                tchs = _chunks(nk)
                v_tm = stile([128, len(tchs), H], "v_att", bufs=1)
                for ci, (off, sz) in enumerate(tchs):
                    ps = psB.tile([128, H], F32, tag="ps")
                    for i in range(2):
                        nc.tensor.matmul(ps[:sz, :], kv_full[:, i, off:off + sz],
                                         wsb["wv"][:, i, :],
                                         start=(i == 0), stop=(i == 1))
                    nc.vector.tensor_tensor(v_tm[:sz, ci, :], ps[:sz, :],
                                            wsb["bv_bc"][:sz, :], ALU.add)
                o_fm = stile([128, 2, nq], f"o_{pfx}", bufs=1)
                inv_sqrt = float(1.0 / np.sqrt(DH))
                if pack:
                    groups = [(0, 3), (1, 3), (2, 2)]
                else:
                    groups = [(b, 1) for b in range(3) for _ in range(1)]
                if pack:
                    for b, nh in groups:
                        m = nh * DH
                        wsc = nh * nq
                        qbd = stile([96, 3 * nq], "qbd")
                        nc.vector.memset(qbd[:m, :wsc], 0.0)
                        for j in range(nh):
                            nc.vector.tensor_copy(
                                qbd[32 * j:32 * j + 32, j * nq:(j + 1) * nq],
                                q_fm[32 * j:32 * j + 32, b, :])
                        pss = psA.tile([wsc, nk], F32, tag="big")
                        for (off, w) in _chunks(nk, 512):
                            nc.tensor.matmul(pss[:, off:off + w],
                                             qbd[:m, :wsc],
                                             k_fm[:m, b, off:off + w],
                                             start=True, stop=True)
                        ee = stile([3 * nq, nk], "sm", bufs=2)
                        ssum = stile([3 * nq, 1], f"as_{pfx}")
                        nc.scalar.activation(ee[:wsc, :], pss[:], AF.Exp,
                                             scale=inv_sqrt,
                                             accum_out=ssum[:wsc, :])
                        nc.vector.reciprocal(ssum[:wsc, :], ssum[:wsc, :])
                        nc.vector.tensor_scalar_mul(ee[:wsc, :], ee[:wsc, :],
                                                    ssum[:wsc, :])
                        at = stile([128, len(tchs), 3 * nq], "at_att")
                        for ci, (off, kj) in enumerate(tchs):
                            ps = psB.tile([128, 128], F32, tag="ps")
                            nc.tensor.transpose(ps[:kj, :wsc],
                                                ee[:wsc, off:off + kj],
                                                ident[:wsc, :wsc])
                            nc.vector.tensor_copy(at[:kj, ci, :wsc],
                                                  ps[:kj, :wsc])
                        pso = psB.tile([96, 3 * nq], F32, tag="ps")
                        for ci, (off, kj) in enumerate(tchs):
                            nc.tensor.matmul(
                                pso[:m, :wsc],
                                v_tm[:kj, ci, 96 * b:96 * b + m],
                                at[:kj, ci, :wsc],
                                start=(ci == 0), stop=(ci == len(tchs) - 1))
                        for j in range(nh):
                            h = 3 * b + j
                            ohb, ohr = divmod(h, 4)
                            nc.vector.tensor_copy(
                                o_fm[ohr * DH:(ohr + 1) * DH, ohb, :],
                                pso[32 * j:32 * j + 32, j * nq:(j + 1) * nq])
                else:
                    for h in range(HEADS):
                        blk, br = divmod(h, 3)
                        rows = slice(br * DH, (br + 1) * DH)
                        pss = psA.tile([nq, nk], F32, tag="big")
                        for (off, w) in _chunks(nk, 512):
                            nc.tensor.matmul(pss[:, off:off + w],
                                             q_fm[rows, blk, :],
                                             k_fm[rows, blk, off:off + w],
                                             start=True, stop=True)
                        ee = stile([nq, nk], "sm", bufs=2)
                        ssum = stile([nq, 1], f"as_{pfx}")
                        nc.scalar.activation(ee[:nq, :], pss[:], AF.Exp,
                                             scale=inv_sqrt,
                                             accum_out=ssum[:nq, :])
                        nc.vector.reciprocal(ssum[:nq, :], ssum[:nq, :])
                        nc.vector.tensor_scalar_mul(ee[:nq, :], ee[:nq, :],
                                                    ssum[:nq, :])
                        at = stile([128, len(tchs), nq], "at_att")
                        for ci, (off, kj) in enumerate(tchs):
                            ps = psB.tile([128, 128], F32, tag="ps")
                            nc.tensor.transpose(ps[:kj, :nq],
                                                ee[:nq, off:off + kj],
                                                ident[:nq, :nq])
                            nc.vector.tensor_copy(at[:kj, ci, :nq],
                                                  ps[:kj, :nq])
                        pso = psB.tile([96, 3 * nq], F32, tag="ps")
                        for ci, (off, kj) in enumerate(tchs):
                            nc.tensor.matmul(
                                pso[:DH, :nq],
                                v_tm[:kj, ci, h * DH:(h + 1) * DH],
                                at[:kj, ci, :nq],
                                start=(ci == 0), stop=(ci == len(tchs) - 1))
                        ohb, ohr = divmod(h, 4)
                        nc.vector.tensor_copy(
                            o_fm[ohr * DH:(ohr + 1) * DH, ohb, :],
                            pso[:DH, :nq])
                ao_fm = stile([128, 2, nq], f"ao_{pfx}")
                fm_linear(ao_fm, o_fm, "wo", "bo", AF.Identity, nq, H)
                li = stile([128, 2, nq], f"li_{pfx}")
                for hb in range(2):
                    nc.vector.tensor_tensor(li[:, hb, :], q_src[:, hb, :],
                                            ao_fm[:, hb, :], ALU.add)
                pss1 = psB.tile([1, nq], F32, tag="ps")
                for hb in range(2):
                    nc.tensor.matmul(pss1[:], wsb["ones2d"][:, 0:1],
                                     li[:, hb, :],
                                     start=(hb == 0), stop=(hb == 1))
                mu = stile([1, nq], f"mu_{pfx}")
                nc.scalar.activation(mu[:], pss1[:], AF.Copy, scale=1.0 / H)
                sq = stile([128, 2, nq], f"sq_{pfx}")
                for hb in range(2):
                    nc.vector.tensor_tensor(sq[:, hb, :], li[:, hb, :],
                                            li[:, hb, :], ALU.mult)
                pss2 = psB.tile([1, nq], F32, tag="ps")
                for hb in range(2):
                    nc.tensor.matmul(pss2[:], wsb["ones2d"][:, 0:1],
                                     sq[:, hb, :],
                                     start=(hb == 0), stop=(hb == 1))
                var = stile([1, nq], f"var_{pfx}")
                mu2 = stile([1, nq], f"mu2_{pfx}")
                nc.scalar.activation(mu2[:], mu[:], AF.Square)
                nc.vector.tensor_scalar_mul(var[:], pss2[:], 1.0 / H)
                nc.vector.tensor_tensor(var[:], var[:], mu2[:], ALU.subtract)
                nc.scalar.activation(var[:], var[:], AF.Sqrt,
                                     bias=wsb["eps"][:1, :])
                nc.vector.reciprocal(var[:], var[:])
                mub = bcast_row(mu[:], nq)
                rsb = bcast_row(var[:], nq)
                lf = stile([128, 2, nq], f"lf_{pfx}")
                for hb in range(2):
                    nc.vector.tensor_tensor(lf[:, hb, :], li[:, hb, :], mub[:],
                                            ALU.subtract)
                    nc.vector.tensor_tensor(lf[:, hb, :], lf[:, hb, :], rsb[:],
                                            ALU.mult)
                    nc.vector.tensor_scalar(lf[:, hb, :], lf[:, hb, :],
                                            wsb[lng][:, hb:hb + 1],
                                            wsb[lnb][:, hb:hb + 1],
                                            ALU.mult, ALU.add)
                lf_ext = stile([nq, H + 1], f"lfe_{pfx}")
                fm_to_tm(lf_ext, lf, nq, 2)
                nc.vector.memset(lf_ext[:, H:H + 1], 1.0)
                psg = psB.tile([B, H + 1], F32, tag="ps")
                nc.tensor.matmul(psg[:], oh_t[:], lf_ext[:],
                                 start=True, stop=True)
                seg = stile([B, H + 1], f"seg_{pfx}")
                nc.vector.tensor_copy(seg[:], psg[:])
                return seg

            with nc.named_scope("attn_l"):
                seg_l = attn_side("l", LLOC, NP, le_fm, pe_full, "lnlg",
                                  "lnlb", sb_in["my_ohl"], pack=True)
            with nc.named_scope("attn_p"):
                seg_p = attn_side("p", PLOC, NL, pe_fm, le_full, "lnpg",
                                  "lnpb", sb_in["my_ohp"], pack=False)

            # ---------- AllReduce readout ----------
            with nc.named_scope("readout"):
                nc.sync.dma_start(ar_i.ap()[:, 0:H + 1], seg_l[:])
                nc.sync.dma_start(ar_i.ap()[:, H + 1:2 * H + 2], seg_p[:])
                nc.gpsimd.collective_compute(
                    "AllReduce", ALU.add, replica_groups=RG,
                    ins=[ar_i.ap().opt()], outs=[ar_o.ap().opt()])
                tot = ptile("tot", [B, 2 * H + 2])
                nc.sync.dma_start(tot[:], ar_o.ap())

            # ---------- decoder (replicated) ----------
            with nc.named_scope("decoder"):
                x_tm = ptile("x_tm", [B, 2 * H])
                cnt = stile([B, 1], "cnt")
                nc.vector.tensor_scalar_max(cnt[:], tot[:, H:H + 1], 1.0)
                nc.vector.reciprocal(cnt[:], cnt[:])
                nc.vector.tensor_scalar_mul(x_tm[:, 0:H], tot[:, 0:H], cnt[:])
                cnt2 = stile([B, 1], "cnt2")
                nc.vector.tensor_scalar_max(cnt2[:], tot[:, 2 * H + 1:2 * H + 2],
                                            1.0)
                nc.vector.reciprocal(cnt2[:], cnt2[:])
                nc.vector.tensor_scalar_mul(x_tm[:, H:2 * H],
                                            tot[:, H + 1:2 * H + 1], cnt2[:])

                x_fm = ptile("x_fm", [128, 4, B])
                tm_to_fm(x_fm, x_tm, B, _chunks(2 * H))

                def bn_leaky(dst, src, mb, gkey, bkey):
                    s1 = stile([128, 1], "bn_s1")
                    nc.vector.tensor_reduce(s1[:], src[:],
                                            mybir.AxisListType.X, ALU.add)
                    sq = stile([128, B], "bn_sq")
                    nc.vector.tensor_tensor(sq[:], src[:], src[:], ALU.mult)
                    s2 = stile([128, 1], "bn_s2")
                    nc.vector.tensor_reduce(s2[:], sq[:],
                                            mybir.AxisListType.X, ALU.add)
                    muv = stile([128, 1], "bn_mu")
                    nc.scalar.activation(muv[:], s1[:], AF.Copy, scale=1.0 / B)
                    mu2 = stile([128, 1], "bn_mu2")
                    nc.scalar.activation(mu2[:], s1[:], AF.Square,
                                         scale=1.0 / B)
                    varv = stile([128, 1], "bn_var")
                    nc.vector.tensor_scalar_mul(varv[:], s2[:], 1.0 / B)
                    nc.vector.tensor_tensor(varv[:], varv[:], mu2[:],
                                            ALU.subtract)
                    nc.scalar.activation(varv[:], varv[:], AF.Sqrt,
                                         bias=wsb["eps"][:, 0:1])
                    nc.vector.reciprocal(varv[:], varv[:])
                    bnv = stile([128, B], "bn_out")
                    nc.vector.tensor_scalar(bnv[:], src[:], muv[:], varv[:],
                                            ALU.subtract, ALU.mult)
                    nc.vector.tensor_scalar(bnv[:], bnv[:],
                                            wsb[gkey][:, mb:mb + 1],
                                            wsb[bkey][:, mb:mb + 1],
                                            ALU.mult, ALU.add)
                    ab = stile([128, B], "bn_abs")
                    nc.scalar.activation(ab[:], bnv[:], AF.Abs, scale=0.45)
                    nc.vector.scalar_tensor_tensor(dst[:], bnv[:], 0.55, ab[:],
                                                   ALU.mult, ALU.add)

                def fc_layer(x_in, KC, Mout, wkey, bkey, gkey, bnbkey,
                             res_from, name):
                    out_t = ptile(name, [128, Mout // 128, B])
                    for mb, (moff, msz) in enumerate(_chunks(Mout)):
                        ps = psB.tile([128, B], F32, tag="ps")
                        for i in range(KC):
                            nc.tensor.matmul(ps[:msz, :],
                                             wsb[wkey][:, i, moff:moff + msz],
                                             x_in[:, i, :],
                                             start=(i == 0), stop=(i == KC - 1))
                        pre = stile([128, B], "fc_pre")
                        nc.scalar.activation(pre[:msz, :], ps[:msz, :],
                                             AF.Identity,
                                             bias=wsb[bkey][:msz, mb:mb + 1])
                        lk = stile([128, B], "fc_lk")
                        bn_leaky(lk, pre, mb, gkey, bnbkey)
                        if res_from is not None:
                            nc.vector.scalar_tensor_tensor(
                                out_t[:, mb, :], res_from[:, mb, :], 0.1, lk[:],
                                ALU.mult, ALU.add)
                        else:
                            nc.vector.tensor_copy(out_t[:, mb, :], lk[:])
                    return out_t

                h1 = fc_layer(x_fm, 4, MLP_HID, "fc1w", "fc1b", "bn1g", "bn1b",
                              x_fm, "h1_fm")
                h2 = fc_layer(h1, 4, MLP_HID, "fc2w", "fc2b", "bn2g", "bn2b",
                              h1, "h2_fm")
                h3pre = fc_layer(h2, 4, MLP_OUT, "fc3w", "fc3b", "bn3g",
                                 "bn3b", None, "h3p_fm")
                psj = psB.tile([128, B], F32, tag="ps")
                for i in range(4):
                    nc.tensor.matmul(psj[:], wsb["pj2w"][:, i, :], h2[:, i, :],
                                     start=(i == 0), stop=(i == 3))
                pj = stile([128, B], "pj_out")
                nc.scalar.activation(pj[:], psj[:], AF.Identity,
                                     bias=wsb["pj2b"][:, 0:1])
                h3 = ptile("h3_fm", [128, 1, B])
                nc.vector.scalar_tensor_tensor(h3[:, 0, :], pj[:], 0.1,
                                               h3pre[:, 0, :], ALU.mult, ALU.add)
                psf = psB.tile([1, B], F32, tag="ps")
                nc.tensor.matmul(psf[:], wsb["fc4w"][:, 0:1], h3[:, 0, :],
                                 start=True, stop=True)
                yout = ptile("yout", [1, B])
                nc.scalar.activation(yout[:], psf[:], AF.Identity,
                                     bias=wsb["fc4b"][:1, 0:1])
                nc.sync.dma_start(out_d.ap().rearrange("a b -> b a"), yout[:])

    nc.compile()
    return nc


def kernel(**inputs) -> np.ndarray:
    wpack, full_in, per_core, alpha = _prep_host(inputs)
    full_shapes = {k: v.shape for k, v in full_in.items()}
    full_shapes.update({k: v.shape for k, v in per_core[0].items()})
    nc = _build(wpack, full_shapes, alpha)
    in_maps = [{**full_in, **pc} for pc in per_core]
    res = run_bass_kernel_spmd(nc, in_maps, core_ids=list(range(NC)))
    return np.asarray(res.results[0]["out"], dtype=np.float32)


if __name__ == "__main__":
    import jax
    jax.config.update("jax_platforms", "cpu")
    import reference as R
    inp = R.setup_inputs()
    ref = np.array(R.reference(**inp))
    act = kernel(**inp)
    rel = np.linalg.norm(act - ref) / np.linalg.norm(ref)
    print("rel fro err:", rel)
